# revision 5
# baseline (speedup 1.0000x reference)
"""KGFIT scoring kernel for 8x Trainium2 NeuronCores (Bass/Tile) — V4.

Strategy (data-parallel, no collectives):
  - Batch rows sharded 8 ways (256 rows/core).
  - Host packs the per-core neg-sample rows (numpy gather of the fp16
    embS=embA+embT table by neg_tails, minus the per-row query q2 =
    2*(h_comb+rel)) into a dense [P, rc, m, d] fp16 array.  Device-side
    indirect gathers are not viable on this part: multi-index SWDGE
    offsets fetch wrong data on HW and CCE accum_op crashes the NEFF, so
    per-row gathers would serialize ~1us of Q7 descriptor work each
    (~130us/core).  Packing host-side also shrinks the upload 12x (no
    replicated 200k x 512 table).
  - Device NEG phase: stream the packed rows (plain DMA) and do the
    |.|-sum reduction, split between DVE (tensor_reduce, 1x) and ACT
    (activation Abs with accum_out); only the mean over m is needed, so
    each chunk reduces to two scalars per partition.
  - Device PAIR phase: 4096x4096 pairwise min-distance, own-512-rows x
    all-4096-cols per core, fp8e4 matmuls with sq_j folded in as a hi/lo
    bf16 K=2 matmul; sq_i (constant per row) is added on host AFTER the
    min.  sq is computed from the fp8-rounded embeddings so
    duplicate-cluster rows still yield ~0 distance, matching the
    reference's diagonal-only masking.
  - Everything else (true/text/intra/parent scores, sqrt/means, final
    assembly) is tiny O(B*D) numpy on host.
"""

import sys
from dataclasses import dataclass

import numpy as np

sys.path.insert(0, "/opt/trn_rl_repo")

RHO, ALPHA, BETA = 0.5, 0.5, 0.5
GAMMA, GAMMA_2 = 12.0, 1.0
LAM1, LAM2 = 1.0, 1.0
EPS = 1e-12
P = 128


@dataclass(frozen=True)
class Cfg:
    nent: int = 200000
    nrel: int = 1000
    nclu: int = 10000
    npar: int = 500
    d: int = 512
    b: int = 2048
    m: int = 64
    ncores: int = 8
    dv: int = 3  # of each 16-row chunk, how many m-rows reduce on DVE

    @property
    def chunks(self):
        """NEG chunk schedule: (rcb, mstart, mwidth, dv).  A small chunk
        leads (streams before clusT so ACT starts early) and a small chunk
        trails (short post-last-load compute)."""
        out = [(1, 48, 8, 4),
               (0, 0, 16, self.dv), (0, 16, 16, self.dv),
               (0, 32, 16, self.dv), (0, 48, 16, self.dv),
               (1, 0, 16, self.dv), (1, 16, 16, self.dv),
               (1, 32, 16, self.dv),
               (1, 56, 8, 4)]
        return out

    @property
    def pc(self):  # batch rows per core
        return self.b // self.ncores

    @property
    def rc(self):  # 128-row chunks of pc
        return self.pc // P

    @property
    def hr(self):  # own pairwise rows per core (h + t)
        return 2 * self.pc

    @property
    def mt(self):  # 128-row tiles of hr
        return self.hr // P

    @property
    def nall(self):  # total pairwise rows/cols
        return 2 * self.b

    @property
    def jbn(self):  # 512-col j blocks
        return (self.nall + 511) // 512

    @property
    def kc(self):  # 128-row K chunks of d
        return self.d // P


REAL = Cfg()

_PROG_CACHE = {}


def build_program(cfg: Cfg):
    from concourse import bacc, tile
    import concourse.bass as bass
    import concourse.mybir as mybir

    f32 = mybir.dt.float32
    f16 = mybir.dt.float16
    bf16 = mybir.dt.bfloat16
    f8 = mybir.dt.float8e4
    AL = mybir.AluOpType
    AX = mybir.AxisListType
    AF = mybir.ActivationFunctionType

    nc = bacc.Bacc(None, target_bir_lowering=False)

    # ---- DRAM tensors
    negdiff_d = nc.dram_tensor(
        "negdiff", [P, cfg.rc, cfg.m, cfg.d], f16, kind="ExternalInput")
    clusT_d = nc.dram_tensor(
        "clusT", [P, cfg.kc, cfg.nall], f8, kind="ExternalInput")
    sqj_d = nc.dram_tensor("sqj", [2, cfg.nall], bf16, kind="ExternalInput")
    ones2_d = nc.dram_tensor("ones2", [2, cfg.hr], bf16, kind="ExternalInput")
    eye_d = nc.dram_tensor("eye", [P, P], f32, kind="ExternalInput")

    # per-chunk partial |.|-sums: [...,0] from DVE, [...,1] from ACT; host
    # sums both per rcb and divides by m (only the mean over m is needed)
    nchunks = len(cfg.chunks)
    onegd_d = nc.dram_tensor("o_negd", [P, nchunks + cfg.mt], f32,
                             kind="ExternalOutput")
    onega_d = nc.dram_tensor("o_nega", [P, nchunks], f32,
                             kind="ExternalOutput")

    with tile.TileContext(nc) as tc:
        with (
            tc.tile_pool(name="const", bufs=1) as const,
            tc.tile_pool(name="work", bufs=1) as work,
            tc.tile_pool(name="small", bufs=1) as small,
            tc.tile_pool(name="psum", bufs=8, space="PSUM") as psum,
        ):
            clusT_sb = const.tile([P, cfg.kc, cfg.nall], f8)
            sqj_sb = const.tile([2, cfg.nall], bf16)
            ones2_sb = const.tile([2, cfg.hr], bf16)
            eye_sb = const.tile([P, P], f32)
            lhs2_sb = const.tile([P, cfg.kc, cfg.hr], f8)
            negaccd = const.tile([P, nchunks + cfg.mt], f32)
            negacca = const.tile([P, nchunks], f32)
            nslot = cfg.jbn + 2
            jmall = const.tile([P, cfg.mt, nslot], f32)

            tiles = []

            def emit_neg_load(step):
                rcb, mstart, mw, dv = cfg.chunks[step]
                at = work.tile([P, mw, cfg.d], f16, tag=f"negload{step}")
                tiles.append(at)
                nc.sync.dma_start(
                    at[:], negdiff_d[:, rcb, mstart:mstart + mw, :])

            def emit_neg_reduce(step):
                rcb, mstart, mw, dv = cfg.chunks[step]
                at = tiles[step]
                # whole-chunk |.|-sum (mean over m only): split DVE/ACT
                nc.vector.tensor_reduce(
                    negaccd[:, step:step + 1], at[:, 0:dv, :],
                    axis=AX.XY, op=AL.add, apply_absolute_value=True)
                nc.scalar.activation(
                    out=at[:, dv:mw, :], in_=at[:, dv:mw, :],
                    func=AF.Abs,
                    accum_out=negacca[:, step:step + 1])

            def emit_pair_block(jb):
                w = min(512, cfg.nall - jb * 512)
                js = slice(jb * 512, jb * 512 + w)
                for mt in range(cfg.mt):
                    ms = slice(mt * P, (mt + 1) * P)
                    pw = psum.tile([P, w], f32, tag="pw", name=f"pw_{jb}_{mt}")
                    for kcb in range(cfg.kc):
                        nc.tensor.matmul(
                            pw[:], lhsT=lhs2_sb[:, kcb, ms],
                            rhs=clusT_sb[:, kcb, js],
                            start=(kcb == 0), stop=False)
                    # += 1*sq_hi + 1*sq_lo  (sq_j term; sq_i added on host)
                    nc.tensor.matmul(
                        pw[:], lhsT=ones2_sb[:, ms],
                        rhs=sqj_sb[:, js],
                        start=False, stop=True)
                    if jb == 0:
                        # diag block: own cols 0..hr-1 (perm puts own first)
                        ysb = small.tile([P, P], f32, tag="ydiag")
                        nc.vector.tensor_add(ysb[:], pw[:, ms], eye_sb[:])
                        nc.vector.tensor_reduce(
                            jmall[:, mt, 0:1], ysb[:], axis=AX.X, op=AL.min)
                        if mt > 0:
                            nc.vector.tensor_reduce(
                                jmall[:, mt, 1:2], pw[:, 0:mt * P],
                                axis=AX.X, op=AL.min)
                        if (mt + 1) * P < w:
                            nc.vector.tensor_reduce(
                                jmall[:, mt, 2:3], pw[:, (mt + 1) * P:w],
                                axis=AX.X, op=AL.min)
                    else:
                        nc.vector.tensor_reduce(
                            jmall[:, mt, 2 + jb:3 + jb], pw[:],
                            axis=AX.X, op=AL.min)

            # ---- emission: the whole pairwise input set loads FIRST (total
            # DMA is fixed, and PE's ~45us of matmuls can only start after
            # clusT lands -- it must not queue behind the fat neg streams);
            # the 9 negdiff chunks stream right behind into dedicated
            # buffers, each feeding its DVE/ACT reduce on arrival.
            half = cfg.nall // 2
            nc.sync.dma_start(sqj_sb[:], sqj_d[:])
            nc.sync.dma_start(ones2_sb[:], ones2_d[:])
            nc.sync.dma_start(clusT_sb[:, :, 0:half], clusT_d[:, :, 0:half])
            # warm the ACT function table before any chunk arrives, so
            # LoadActFuncSet doesn't serialize into the input-paced abs chain
            actwarm = small.tile([P, 1], f32, tag="actwarm")
            nc.vector.memset(actwarm[:], 0.0)
            nc.scalar.activation(out=actwarm[:], in_=actwarm[:], func=AF.Abs)
            nc.vector.memset(jmall[:], 1e30)
            # lhsT = -2 * own columns of clusT (exact x2 scaling in fp8)
            nc.vector.tensor_scalar_mul(
                lhs2_sb[:], clusT_sb[:, :, 0:cfg.hr], -2.0)
            emit_neg_load(0)
            nc.sync.dma_start(eye_sb[:], eye_d[:])
            emit_neg_load(1)
            nc.sync.dma_start(clusT_sb[:, :, half:], clusT_d[:, :, half:])
            for step in range(2, nchunks):
                emit_neg_load(step)

            emit_neg_reduce(0)
            for jb in range(cfg.jbn - 1):
                emit_pair_block(jb)
                emit_neg_reduce(jb + 1)
            emit_pair_block(cfg.jbn - 1)
            for mt in range(cfg.mt):
                nc.vector.tensor_reduce(
                    negaccd[:, nchunks + mt:nchunks + mt + 1], jmall[:, mt, :],
                    axis=AX.X, op=AL.min)
            emit_neg_reduce(cfg.jbn)
            nc.sync.dma_start(onegd_d[:], negaccd[:])
            nc.sync.dma_start(onega_d[:], negacca[:])

    nc.compile()
    return nc


def _chunked(x, nch):
    """[N, ...] -> [128, nch, ...] with row r at [r%128, r//128]."""
    n = x.shape[0]
    assert n == nch * P
    return np.ascontiguousarray(x.reshape(nch, P, *x.shape[1:]).transpose(
        1, 0, *range(2, x.ndim + 1)))


def _unchunk(x):
    """[128, nch, ...] -> [nch*128, ...] inverting _chunked."""
    return np.ascontiguousarray(
        x.transpose(1, 0, *range(2, x.ndim))).reshape(-1, *x.shape[2:])


def make_in_maps(cfg: Cfg, sample, neg_tails, cluster_assign, parent_assign,
                 relation_embedding, entity_embedding_init,
                 entity_text_embeddings, cluster_emb, parent_emb):
    import ml_dtypes
    f4 = np.float32
    f16 = np.float16
    bf = ml_dtypes.bfloat16
    f8 = ml_dtypes.float8_e4m3
    sample = np.asarray(sample)
    neg_tails = np.asarray(neg_tails)
    cluster_assign = np.asarray(cluster_assign)
    parent_assign = np.asarray(parent_assign)
    relation_embedding = np.asarray(relation_embedding, dtype=f4)
    embA = np.asarray(entity_embedding_init, dtype=f4)
    embT = np.asarray(entity_text_embeddings, dtype=f4)
    embS = (embA + embT).astype(f16)
    cluster_emb = np.asarray(cluster_emb, dtype=f4)
    parent_emb = np.asarray(parent_emb, dtype=f4)

    h_all = sample[:, 0].astype(np.int64)
    r_all = (sample[:, 1] % cfg.nrel).astype(np.int64)
    t_all = sample[:, 2].astype(np.int64)
    cid_all = cluster_assign[np.concatenate([h_all, t_all])]
    clus = cluster_emb[cid_all]                       # [2B, d] f32
    clus_f8 = clus.astype(f8)
    clus_f832 = clus_f8.astype(f4)
    sq_all = np.sum(clus_f832 * clus_f832, axis=1, dtype=f4)  # from fp8 vals
    sq_hi = sq_all.astype(bf)
    sq_lo = (sq_all - sq_hi.astype(f4)).astype(bf)
    eye = (np.eye(P) * 1e9).astype(f4)

    # host-side per-row terms
    h_comb = RHO * embA[h_all] + (1.0 - RHO) * embT[h_all]     # [B,d] f32
    t_comb = RHO * embA[t_all] + (1.0 - RHO) * embT[t_all]
    rel = relation_embedding[r_all]
    q2 = 2.0 * (h_comb + rel)                                  # [B,d]
    true_s = (GAMMA - np.abs(h_comb + rel - t_comb).sum(axis=1)).astype(f4)
    hd = np.sqrt(np.sum((h_comb - embT[h_all]) ** 2, axis=1) + EPS).astype(f4)
    td = np.sqrt(np.sum((t_comb - embT[t_all]) ** 2, axis=1) + EPS).astype(f4)
    combs = np.concatenate([h_comb, t_comb], axis=0)           # [2B,d]
    intra_d = np.sqrt(np.sum((combs - clus) ** 2, axis=1) + EPS).astype(f4)
    pars_all = parent_emb[parent_assign[cid_all]]
    par_d = np.sqrt(np.sum((clus - pars_all) ** 2, axis=1) + EPS).astype(f4)

    in_maps, aux = [], []
    for k in range(cfg.ncores):
        bs = slice(k * cfg.pc, (k + 1) * cfg.pc)
        neg = neg_tails[bs].astype(np.int64)                   # [pc, m]
        # packed neg rows minus the per-row query, fp16 (device streams
        # these and reduces |.|; fp16 rounding matches the emulated error)
        rows = embS[neg].astype(f4)                            # [pc, m, d]
        rows -= q2[bs][:, None, :]
        negdiff = _chunked(rows.astype(f16), cfg.rc)           # [P,rc,m,d]

        own = np.concatenate([np.arange(k * cfg.pc, (k + 1) * cfg.pc),
                              np.arange(cfg.b + k * cfg.pc,
                                        cfg.b + (k + 1) * cfg.pc)])
        mask = np.ones(cfg.nall, dtype=bool)
        mask[own] = False
        perm = np.concatenate([own, np.nonzero(mask)[0]])
        clusP = clus_f8[perm]                                  # [nall, d] fp8
        clusT_in = np.ascontiguousarray(
            clusP.T.reshape(cfg.kc, P, cfg.nall).transpose(1, 0, 2))
        sqj = np.stack([sq_hi[perm], sq_lo[perm]])             # [2, nall] bf16
        ones2 = np.ones((2, cfg.hr), dtype=bf)

        in_maps.append({
            "negdiff": negdiff,
            "clusT": clusT_in,
            "sqj": sqj,
            "ones2": ones2,
            "eye": eye,
        })
        aux.append({"sq_own": sq_all[own]})
    host = {
        "true_s": true_s, "hd": hd, "td": td,
        "intra_d": intra_d, "par_d": par_d,
    }
    return in_maps, (aux, host)


def assemble(cfg: Cfg, results, aux_host):
    aux, host = aux_host
    f4 = np.float32
    mean_neg, inter_d = [], []
    chunks = cfg.chunks
    for k in range(cfg.ncores):
        r = results[k]
        od, oa = r["o_negd"], r["o_nega"]  # [P, nchunks(+mt)] partials
        raw = np.zeros((P, cfg.rc), f4)
        for i, (rcb, _, _, _) in enumerate(chunks):
            raw[:, rcb] += od[:, i] + oa[:, i]
        raw_mean = _unchunk(raw[:, :, None])[:, 0] / cfg.m  # [pc]
        mean_neg.append((GAMMA - 0.5 * raw_mean).astype(f4))
        nch = len(chunks)
        inter_min = _unchunk(od[:, nch:nch + cfg.mt][:, :, None])[:, 0]
        d2 = inter_min + aux[k]["sq_own"]
        inter_d.append(np.sqrt(np.maximum(d2, EPS), dtype=f4))

    intra_loss = host["intra_d"].mean(dtype=f4)
    par_loss = host["par_d"].mean(dtype=f4)
    inter_loss = np.concatenate(inter_d).mean(dtype=f4)
    hier = intra_loss - LAM1 * inter_loss + LAM2 * par_loss

    mean_neg = np.concatenate(mean_neg)
    score = (-ALPHA * hier - BETA * (host["hd"] + host["td"])
             - GAMMA_2 * (host["true_s"] - mean_neg)).astype(f4)
    return score


def run_on_device(cfg: Cfg, in_maps, trace=False):
    from concourse.bass_utils import run_bass_kernel_spmd
    key = cfg
    if key not in _PROG_CACHE:
        _PROG_CACHE[key] = build_program(cfg)
    nc = _PROG_CACHE[key]
    res = run_bass_kernel_spmd(
        nc, in_maps, core_ids=list(range(cfg.ncores)), trace=trace)
    return res


def kernel(**inputs):
    cfg = REAL
    in_maps, aux_host = make_in_maps(cfg, **inputs)
    res = run_on_device(cfg, in_maps)
    return assemble(cfg, res.results, aux_host)
